# revision 25
# baseline (speedup 1.0000x reference)
"""Trainium2 Bass kernel for nn_Decoder_78864189489953.

Pointer-generator decoder step: Bahdanau coverage attention + 1-step LSTM +
pointer copy distribution  p_copy = attn_weights @ src_map.

Sharding: batch B=8 across the 8 NeuronCores (one batch element per core).
src_map is 8x512x32000 f32 = 524 MB -- the kernel is memory bound on
streaming each core's 65.5 MB slice; everything else hides under that DMA.

Dataflow per core (all input math on device; host only slices/transposes/
gathers):
  - attention/LSTM in fp32 on TensorE/VectorE/ScalarE (errors ~3e-6)
  - p_copy stream: src DMA'd fp32->fp16 in-flight (SWDGE cast; HBM read
    bytes unchanged), fp16 matvec on TensorE accumulating fp32 in PSUM
    (p_copy rel err ~6e-5). Three DMA rings: params on sync, src stream
    on gpsimd, stores on scalar.

Self-contained: hardcodes shapes; imports concourse from the container
install.
"""

import os
import sys

import numpy as np

for _p in ("/opt/trn_rl_repo",):
    if _p not in sys.path and os.path.isdir(_p):
        sys.path.insert(0, _p)

B, S, V = 8, 512, 32000
H, E = 256, 128
EPS = 1e-6

VW = 2000   # src v-chunk width per DMA tile ([128, 4, VW] f32 = 4 MB)
NW = 500    # psum slice width per matmul (<=512 fp32 / PSUM bank)
NSC = S // 128          # 4 s-chunks
NVC = V // VW           # 16 v-chunks
NNW = VW // NW          # 4 psum slices per v-chunk
HEAT_PER_CHUNK = 12     # junk matmuls between chunks to keep PE at 2.4 GHz

_cache = {}


def _build_nc():
    import concourse.mybir as mybir
    import concourse.tile as tile
    from concourse import bacc

    f32 = mybir.dt.float32
    AF = mybir.ActivationFunctionType

    nc = bacc.Bacc("TRN2", target_bir_lowering=False, debug=False)

    def din(name, shape, dt=f32):
        return nc.dram_tensor(name, shape, dt, kind="ExternalInput").ap()

    def dout(name, shape):
        return nc.dram_tensor(name, shape, f32, kind="ExternalOutput").ap()

    # per-core inputs (host pre-tiled into [128, chunk, free] layouts)
    src = din("src", [S, V])            # [512, 32000] this core's src_map slice
    enc = din("enc", [128, NSC, 2 * H])     # enc rows s-chunked
    augrhs = din("augrhs", [2, S])          # [coverage ; ones]
    h0col = din("h0col", [128, 2])          # h0 as column chunks
    c0row = din("c0row", [1, H])
    embcol = din("embcol", [128, 1])        # gathered embedding row as column
    # replicated parameters
    wencT = din("wencT", [128, NSC, H])     # attn_W[:,256:768].T  d-chunked
    wdecT = din("wdecT", [128, 2, H])       # attn_W[:,0:256].T    k-chunked
    auglhsT = din("auglhsT", [2, H])        # [sum(attn_W[:,768:]) ; attn_b]
    vcol = din("vcol", [128, 2])            # attn_v column chunks
    wihT = din("wihT", [128, 5, 4 * H])     # W_ih.T k-chunked
    whhT = din("whhT", [128, 2, 4 * H])     # W_hh.T k-chunked
    brow = din("brow", [1, 4 * H])          # b_ih + b_hh

    pcopy_o = dout("pcopy", [1, V])
    attnw_o = dout("attnw", [1, S])
    newcov_o = dout("newcov", [1, S])
    hnew_o = dout("hnew", [1, H])
    cnew_o = dout("cnew", [1, H])

    with tile.TileContext(nc) as tc:
        with (
            tc.tile_pool(name="singles", bufs=1) as sg,
            tc.tile_pool(name="srcpool", bufs=7) as sp,
            tc.tile_pool(name="prowpool", bufs=2) as op,
            tc.tile_pool(name="ps_et", bufs=2, space="PSUM") as ps_et,
            tc.tile_pool(name="ps_col", bufs=1, space="PSUM") as ps_col,
            tc.tile_pool(name="ps_row", bufs=2, space="PSUM") as ps_row,
            tc.tile_pool(name="ps_p", bufs=3, space="PSUM") as ps_p,
        ):
            # ---- load small tensors (sync ring, ahead of src stream) ----
            enc_sb = sg.tile([128, NSC, 2 * H], f32)
            augrhs_sb = sg.tile([2, S], f32)
            h0col_sb = sg.tile([128, 2], f32)
            c0row_sb = sg.tile([1, H], f32)
            wencT_sb = sg.tile([128, NSC, H], f32)
            wdecT_sb = sg.tile([128, 2, H], f32)
            auglhsT_sb = sg.tile([2, H], f32)
            vcol_sb = sg.tile([128, 2], f32)
            wihT_sb = sg.tile([128, 5, 4 * H], f32)
            whhT_sb = sg.tile([128, 2, 4 * H], f32)
            brow_sb = sg.tile([1, 4 * H], f32)
            xcol_sb = sg.tile([128, 5], f32)   # [emb ; ctx chunks] columns

            for dst, src_ap in (
                (enc_sb, enc), (augrhs_sb, augrhs),
                (h0col_sb, h0col), (c0row_sb, c0row),
                (wencT_sb, wencT), (wdecT_sb, wdecT),
                (auglhsT_sb, auglhsT), (vcol_sb, vcol),
                (wihT_sb, wihT), (whhT_sb, whhT), (brow_sb, brow),
            ):
                nc.sync.dma_start(out=dst, in_=src_ap)
            nc.sync.dma_start(out=xcol_sb[:, 0:1], in_=embcol)

            ones11 = sg.tile([1, 1], f32)
            nc.vector.memset(ones11, 1.0)

            # ---- transpose enc on PE -> encT_sb[d_chunk, s] ----
            from concourse.masks import make_identity
            identity = sg.tile([128, 128], f32)
            make_identity(nc, identity)
            encT_sb = sg.tile([128, NSC, S], f32)
            for sj in range(NSC):
                for kc in range(NSC):
                    ptr = ps_et.tile([128, 128], f32, tag="et")
                    nc.tensor.transpose(
                        ptr, enc_sb[:, sj, kc * 128:(kc + 1) * 128], identity)
                    nc.vector.tensor_copy(
                        encT_sb[:, kc, sj * 128:(sj + 1) * 128], ptr)

            # ---- attention: eT[h,s] = tanh(Wenc-part + cov/bias-part + dec) --
            dech_sb = sg.tile([128, 2], f32)
            eT_sb = sg.tile([128, 2, S], f32)
            for hc in range(2):
                hs = slice(hc * 128, (hc + 1) * 128)
                pdec = ps_col.tile([128, 1], f32, tag="col")
                for kc in range(2):
                    nc.tensor.matmul(
                        pdec, lhsT=wdecT_sb[:, kc, hs], rhs=h0col_sb[:, kc:kc + 1],
                        start=(kc == 0), stop=(kc == 1))
                nc.vector.tensor_copy(dech_sb[:, hc:hc + 1], pdec)

                pet = ps_et.tile([128, S], f32, tag="et")
                for kc in range(NSC):
                    nc.tensor.matmul(
                        pet, lhsT=wencT_sb[:, kc, hs], rhs=encT_sb[:, kc, :],
                        start=(kc == 0), stop=False)
                nc.tensor.matmul(
                    pet, lhsT=auglhsT_sb[:, hs], rhs=augrhs_sb,
                    start=False, stop=True)
                nc.scalar.activation(
                    out=eT_sb[:, hc, :], in_=pet, func=AF.Tanh,
                    bias=dech_sb[:, hc:hc + 1], scale=1.0)

            # ---- scores + softmax (row layout [1, S]) ----
            psc = ps_row.tile([1, S], f32, tag="row")
            for hc in range(2):
                nc.tensor.matmul(
                    psc, lhsT=vcol_sb[:, hc:hc + 1], rhs=eT_sb[:, hc, :],
                    start=(hc == 0), stop=(hc == 1))
            mx = sg.tile([1, 1], f32)
            negmx = sg.tile([1, 1], f32)
            ssum = sg.tile([1, 1], f32)
            rinv = sg.tile([1, 1], f32)
            w_row = sg.tile([1, S], f32)
            newcov_row = sg.tile([1, S], f32)
            nc.vector.reduce_max(mx, psc, axis=mybir.AxisListType.X)
            nc.vector.tensor_scalar_mul(negmx, mx, -1.0)
            nc.scalar.activation(
                out=w_row, in_=psc, func=AF.Exp,
                bias=negmx[0:1, 0:1], scale=1.0, accum_out=ssum)
            nc.vector.reciprocal(rinv, ssum)
            nc.vector.tensor_scalar_mul(w_row, w_row, rinv[0:1, 0:1])
            nc.vector.tensor_add(newcov_row, w_row, augrhs_sb[0:1, :])
            nc.scalar.dma_start(out=attnw_o, in_=w_row)
            nc.scalar.dma_start(out=newcov_o, in_=newcov_row)

            # ---- w as columns (via K=1 transpose-matmuls) ----
            wcol_sb = sg.tile([128, NSC], f32)
            for j in range(NSC):
                pw = ps_col.tile([128, 1], f32, tag="col")
                nc.tensor.matmul(
                    pw, lhsT=w_row[0:1, j * 128:(j + 1) * 128], rhs=ones11,
                    start=True, stop=True)
                nc.vector.tensor_copy(wcol_sb[:, j:j + 1], pw)
            # fp16 copy for the full-rate p_copy matmuls
            f16 = mybir.dt.float16
            wcolh_sb = sg.tile([128, NSC], f16)
            nc.vector.tensor_copy(wcolh_sb, wcol_sb)

            # ---- context + its column form ----
            pctx = ps_row.tile([1, 2 * H], f32, tag="row")
            for j in range(NSC):
                nc.tensor.matmul(
                    pctx, lhsT=wcol_sb[:, j:j + 1], rhs=enc_sb[:, j, :],
                    start=(j == 0), stop=(j == NSC - 1))
            ctx_row = sg.tile([1, 2 * H], f32)
            nc.vector.tensor_copy(ctx_row, pctx)
            for j in range(NSC):
                pcj = ps_col.tile([128, 1], f32, tag="col")
                nc.tensor.matmul(
                    pcj, lhsT=ctx_row[0:1, j * 128:(j + 1) * 128], rhs=ones11,
                    start=True, stop=True)
                nc.vector.tensor_copy(xcol_sb[:, 1 + j:2 + j], pcj)

            # ---- LSTM gates (bias added in-PSUM via a K=1 matmul) ----
            pgs = []
            for ng in range(2):
                ns = slice(ng * 512, (ng + 1) * 512)
                pg = ps_row.tile([1, 512], f32, tag="row")
                for kc in range(5):
                    nc.tensor.matmul(
                        pg, lhsT=xcol_sb[:, kc:kc + 1], rhs=wihT_sb[:, kc, ns],
                        start=(kc == 0), stop=False)
                for kh in range(2):
                    nc.tensor.matmul(
                        pg, lhsT=h0col_sb[:, kh:kh + 1], rhs=whhT_sb[:, kh, ns],
                        start=False, stop=False)
                nc.tensor.matmul(
                    pg, lhsT=ones11, rhs=brow_sb[0:1, ns],
                    start=False, stop=True)
                pgs.append(pg)

            si = sg.tile([1, H], f32)
            sf = sg.tile([1, H], f32)
            tg = sg.tile([1, H], f32)
            so = sg.tile([1, H], f32)
            cnew_row = sg.tile([1, H], f32)
            nc.scalar.activation(out=si, in_=pgs[0][0:1, 0:H], func=AF.Sigmoid)
            nc.scalar.activation(out=sf, in_=pgs[0][0:1, H:2 * H], func=AF.Sigmoid)
            nc.scalar.activation(out=tg, in_=pgs[1][0:1, 0:H], func=AF.Tanh)
            nc.scalar.activation(out=so, in_=pgs[1][0:1, H:2 * H], func=AF.Sigmoid)
            nc.vector.tensor_mul(sf, sf, c0row_sb)
            nc.vector.tensor_mul(si, si, tg)
            nc.vector.tensor_add(cnew_row, sf, si)
            nc.scalar.dma_start(out=cnew_o, in_=cnew_row)
            nc.scalar.activation(out=tg, in_=cnew_row, func=AF.Tanh)
            nc.vector.tensor_mul(so, so, tg)
            nc.scalar.dma_start(out=hnew_o, in_=so)

            # ---- p_copy = w @ src: 65.5 MB fp32 read, cast to fp16 in-DMA
            # (SWDGE/gpsimd ring), fp16 matmuls at full PE rate ----
            src_t = src.rearrange("(c p) v -> p c v", p=128)
            chunks = [(i * VW, VW) for i in range(NVC - 1)]
            tw = VW // 4
            chunks += [((NVC - 1) * VW + i * tw, tw) for i in range(4)]
            for off, width in chunks:
                vs = slice(off, off + width)
                t = sp.tile([128, NSC, VW], f16, tag="src")
                nc.gpsimd.dma_start(out=t[:, :, 0:width], in_=src_t[:, :, vs])
                prow = op.tile([1, VW], f32, tag="prow")
                for n in range(width // NW):
                    pt = ps_p.tile([1, NW], f32, tag="pc")
                    for sc in range(NSC):
                        nc.tensor.matmul(
                            pt, lhsT=wcolh_sb[:, sc:sc + 1],
                            rhs=t[:, sc, n * NW:(n + 1) * NW],
                            start=(sc == 0), stop=(sc == NSC - 1))
                    nc.vector.tensor_scalar_add(
                        prow[0:1, n * NW:(n + 1) * NW], pt, EPS)
                nc.scalar.dma_start(out=pcopy_o[0:1, vs], in_=prow[0:1, 0:width])

    nc.compile()
    return nc


def _tile4(x):
    """[C*128, N] -> [128, C, N] partition-chunked layout."""
    c = x.shape[0] // 128
    return np.ascontiguousarray(
        x.reshape(c, 128, *x.shape[1:]).transpose(1, 0, *range(2, x.ndim + 1)))


def _prep(inputs):
    f = lambda k: np.ascontiguousarray(np.asarray(inputs[k], dtype=np.float32))
    prev_word = np.asarray(inputs["prev_word"])
    state_h, state_c = f("state_h"), f("state_c")
    enc_all, cov_all, src_all = f("encoder_outputs"), f("coverage"), f("src_map")
    embed = f("embed")
    attn_W, attn_b, attn_v = f("attn_W"), f("attn_b"), f("attn_v")
    W_ih, W_hh = f("W_ih"), f("W_hh")
    b_ih, b_hh = f("b_ih"), f("b_hh")

    # replicated parameter repacks
    wencT = _tile4(np.ascontiguousarray(attn_W[:, H:3 * H].T))    # [128,4,256]
    wdecT = _tile4(np.ascontiguousarray(attn_W[:, 0:H].T))        # [128,2,256]
    auglhsT = np.ascontiguousarray(
        np.stack([attn_W[:, 3 * H:4 * H].sum(axis=1), attn_b]))   # [2,256]
    vcol = np.ascontiguousarray(attn_v.reshape(2, 128).T)         # [128,2]
    wihT = _tile4(np.ascontiguousarray(W_ih.T))                   # [128,5,1024]
    whhT = _tile4(np.ascontiguousarray(W_hh.T))                   # [128,2,1024]
    brow = np.ascontiguousarray((b_ih + b_hh).reshape(1, 4 * H))

    ones_s = np.ones((S,), np.float32)
    in_maps = []
    for b in range(B):
        enc_b = enc_all[b]
        in_maps.append({
            "src": src_all[b],
            "enc": _tile4(enc_b),
            "augrhs": np.ascontiguousarray(np.stack([cov_all[b], ones_s])),
            "h0col": np.ascontiguousarray(state_h[0, b].reshape(2, 128).T),
            "c0row": np.ascontiguousarray(state_c[0, b].reshape(1, H)),
            "embcol": np.ascontiguousarray(
                embed[int(prev_word[b, 0])].reshape(128, 1)),
            "wencT": wencT, "wdecT": wdecT, "auglhsT": auglhsT,
            "vcol": vcol, "wihT": wihT, "whhT": whhT, "brow": brow,
        })
    return in_maps


def _run(in_maps, **kw):
    from concourse import bass_utils
    if "nc" not in _cache:
        _cache["nc"] = _build_nc()
    return bass_utils.run_bass_kernel_spmd(
        _cache["nc"], in_maps, core_ids=list(range(B)), **kw)


def kernel(**inputs):
    res = _run(_prep(inputs))
    outs = res.results
    p_copy = np.concatenate([o["pcopy"] for o in outs], axis=0)
    attnw = np.concatenate([o["attnw"] for o in outs], axis=0)
    newcov = np.concatenate([o["newcov"] for o in outs], axis=0)
    hnew = np.concatenate([o["hnew"] for o in outs], axis=0)[None]
    cnew = np.concatenate([o["cnew"] for o in outs], axis=0)[None]
    return p_copy, attnw, newcov, (hnew, cnew)


# revision 27
# speedup vs baseline: 1.0112x; 1.0112x over previous
"""Trainium2 Bass kernel for nn_Decoder_78864189489953.

Pointer-generator decoder step: Bahdanau coverage attention + 1-step LSTM +
pointer copy distribution  p_copy = attn_weights @ src_map.

Sharding: batch B=8 across the 8 NeuronCores (one batch element per core).
src_map is 8x512x32000 f32 = 524 MB -- the kernel is memory bound on
streaming each core's 65.5 MB slice; everything else hides under that DMA.

Dataflow per core (all input math on device; host only slices/transposes/
gathers):
  - attention/LSTM in fp32 on TensorE/VectorE/ScalarE (errors ~3e-6)
  - p_copy stream: src DMA'd fp32->fp16 in-flight (SWDGE cast; HBM read
    bytes unchanged), fp16 matvec on TensorE accumulating fp32 in PSUM
    (p_copy rel err ~6e-5). Three DMA rings: params on sync, src stream
    on gpsimd, stores on scalar.

Self-contained: hardcodes shapes; imports concourse from the container
install.
"""

import os
import sys

import numpy as np

for _p in ("/opt/trn_rl_repo",):
    if _p not in sys.path and os.path.isdir(_p):
        sys.path.insert(0, _p)

B, S, V = 8, 512, 32000
H, E = 256, 128
EPS = 1e-6

VW = 2000   # src v-chunk width per DMA tile ([128, 4, VW] f32 = 4 MB)
NW = 500    # psum slice width per matmul (<=512 fp32 / PSUM bank)
NSC = S // 128          # 4 s-chunks
NVC = V // VW           # 16 v-chunks
NNW = VW // NW          # 4 psum slices per v-chunk
HEAT_PER_CHUNK = 12     # junk matmuls between chunks to keep PE at 2.4 GHz

_cache = {}


def _build_nc():
    import concourse.mybir as mybir
    import concourse.tile as tile
    from concourse import bacc

    f32 = mybir.dt.float32
    AF = mybir.ActivationFunctionType

    nc = bacc.Bacc("TRN2", target_bir_lowering=False, debug=False)

    def din(name, shape, dt=f32):
        return nc.dram_tensor(name, shape, dt, kind="ExternalInput").ap()

    def dout(name, shape):
        return nc.dram_tensor(name, shape, f32, kind="ExternalOutput").ap()

    # per-core inputs (host pre-tiled into [128, chunk, free] layouts)
    src = din("src", [S, V])            # [512, 32000] this core's src_map slice
    enc = din("enc", [128, NSC, 2 * H])     # enc rows s-chunked
    augrhs = din("augrhs", [2, S])          # [coverage ; ones]
    h0col = din("h0col", [128, 2])          # h0 as column chunks
    c0row = din("c0row", [1, H])
    embcol = din("embcol", [128, 1])        # gathered embedding row as column
    # replicated parameters
    wencT = din("wencT", [128, NSC, H])     # attn_W[:,256:768].T  d-chunked
    wdecT = din("wdecT", [128, 2, H])       # attn_W[:,0:256].T    k-chunked
    auglhsT = din("auglhsT", [2, H])        # [sum(attn_W[:,768:]) ; attn_b]
    vcol = din("vcol", [128, 2])            # attn_v column chunks
    wihT = din("wihT", [128, 5, 4 * H])     # W_ih.T k-chunked
    whhT = din("whhT", [128, 2, 4 * H])     # W_hh.T k-chunked
    brow = din("brow", [1, 4 * H])          # b_ih + b_hh

    pcopy_o = dout("pcopy", [1, V])
    attnw_o = dout("attnw", [1, S])
    newcov_o = dout("newcov", [1, S])
    hnew_o = dout("hnew", [1, H])
    cnew_o = dout("cnew", [1, H])

    with tile.TileContext(nc) as tc:
        with (
            tc.tile_pool(name="singles", bufs=1) as sg,
            tc.tile_pool(name="srcpool", bufs=6) as sp,
            tc.tile_pool(name="prowpool", bufs=2) as op,
            tc.tile_pool(name="ps_et", bufs=2, space="PSUM") as ps_et,
            tc.tile_pool(name="ps_col", bufs=1, space="PSUM") as ps_col,
            tc.tile_pool(name="ps_row", bufs=2, space="PSUM") as ps_row,
            tc.tile_pool(name="ps_p", bufs=3, space="PSUM") as ps_p,
        ):
            # ---- load small tensors (sync ring, ahead of src stream) ----
            enc_sb = sg.tile([128, NSC, 2 * H], f32)
            augrhs_sb = sg.tile([2, S], f32)
            h0col_sb = sg.tile([128, 2], f32)
            c0row_sb = sg.tile([1, H], f32)
            wencT_sb = sg.tile([128, NSC, H], f32)
            wdecT_sb = sg.tile([128, 2, H], f32)
            auglhsT_sb = sg.tile([2, H], f32)
            vcol_sb = sg.tile([128, 2], f32)
            wihT_sb = sg.tile([128, 5, 4 * H], f32)
            whhT_sb = sg.tile([128, 2, 4 * H], f32)
            brow_sb = sg.tile([1, 4 * H], f32)
            xcol_sb = sg.tile([128, 5], f32)   # [emb ; ctx chunks] columns

            for dst, src_ap in (
                (enc_sb, enc), (augrhs_sb, augrhs),
                (h0col_sb, h0col), (c0row_sb, c0row),
                (wencT_sb, wencT), (wdecT_sb, wdecT),
                (auglhsT_sb, auglhsT), (vcol_sb, vcol),
                (wihT_sb, wihT), (whhT_sb, whhT), (brow_sb, brow),
            ):
                nc.sync.dma_start(out=dst, in_=src_ap)
            nc.sync.dma_start(out=xcol_sb[:, 0:1], in_=embcol)

            ones11 = sg.tile([1, 1], f32)
            nc.vector.memset(ones11, 1.0)

            # ---- transpose enc on PE -> encT_sb[d_chunk, s] ----
            from concourse.masks import make_identity
            identity = sg.tile([128, 128], f32)
            make_identity(nc, identity)
            encT_sb = sg.tile([128, NSC, S], f32)
            for sj in range(NSC):
                for kc in range(NSC):
                    ptr = ps_et.tile([128, 128], f32, tag="et")
                    nc.tensor.transpose(
                        ptr, enc_sb[:, sj, kc * 128:(kc + 1) * 128], identity)
                    nc.vector.tensor_copy(
                        encT_sb[:, kc, sj * 128:(sj + 1) * 128], ptr)

            # ---- attention: eT[h,s] = tanh(Wenc-part + cov/bias-part + dec) --
            dech_sb = sg.tile([128, 2], f32)
            eT_sb = sg.tile([128, 2, S], f32)
            for hc in range(2):
                hs = slice(hc * 128, (hc + 1) * 128)
                pdec = ps_col.tile([128, 1], f32, tag="col")
                for kc in range(2):
                    nc.tensor.matmul(
                        pdec, lhsT=wdecT_sb[:, kc, hs], rhs=h0col_sb[:, kc:kc + 1],
                        start=(kc == 0), stop=(kc == 1))
                nc.vector.tensor_copy(dech_sb[:, hc:hc + 1], pdec)

                pet = ps_et.tile([128, S], f32, tag="et")
                for kc in range(NSC):
                    nc.tensor.matmul(
                        pet, lhsT=wencT_sb[:, kc, hs], rhs=encT_sb[:, kc, :],
                        start=(kc == 0), stop=False)
                nc.tensor.matmul(
                    pet, lhsT=auglhsT_sb[:, hs], rhs=augrhs_sb,
                    start=False, stop=True)
                nc.scalar.activation(
                    out=eT_sb[:, hc, :], in_=pet, func=AF.Tanh,
                    bias=dech_sb[:, hc:hc + 1], scale=1.0)

            # ---- scores + softmax (row layout [1, S]) ----
            psc = ps_row.tile([1, S], f32, tag="row")
            for hc in range(2):
                nc.tensor.matmul(
                    psc, lhsT=vcol_sb[:, hc:hc + 1], rhs=eT_sb[:, hc, :],
                    start=(hc == 0), stop=(hc == 1))
            mx = sg.tile([1, 1], f32)
            negmx = sg.tile([1, 1], f32)
            ssum = sg.tile([1, 1], f32)
            rinv = sg.tile([1, 1], f32)
            w_row = sg.tile([1, S], f32)
            newcov_row = sg.tile([1, S], f32)
            nc.vector.reduce_max(mx, psc, axis=mybir.AxisListType.X)
            nc.vector.tensor_scalar_mul(negmx, mx, -1.0)
            nc.scalar.activation(
                out=w_row, in_=psc, func=AF.Exp,
                bias=negmx[0:1, 0:1], scale=1.0, accum_out=ssum)
            nc.vector.reciprocal(rinv, ssum)
            nc.vector.tensor_scalar_mul(w_row, w_row, rinv[0:1, 0:1])
            nc.vector.tensor_add(newcov_row, w_row, augrhs_sb[0:1, :])
            nc.scalar.dma_start(out=attnw_o, in_=w_row)
            nc.scalar.dma_start(out=newcov_o, in_=newcov_row)

            # ---- w as columns (via K=1 transpose-matmuls) ----
            wcol_sb = sg.tile([128, NSC], f32)
            for j in range(NSC):
                pw = ps_col.tile([128, 1], f32, tag="col")
                nc.tensor.matmul(
                    pw, lhsT=w_row[0:1, j * 128:(j + 1) * 128], rhs=ones11,
                    start=True, stop=True)
                nc.vector.tensor_copy(wcol_sb[:, j:j + 1], pw)
            # fp16 copy for the full-rate p_copy matmuls
            f16 = mybir.dt.float16
            wcolh_sb = sg.tile([128, NSC], f16)
            nc.vector.tensor_copy(wcolh_sb, wcol_sb)

            # ---- context + its column form ----
            pctx = ps_row.tile([1, 2 * H], f32, tag="row")
            for j in range(NSC):
                nc.tensor.matmul(
                    pctx, lhsT=wcol_sb[:, j:j + 1], rhs=enc_sb[:, j, :],
                    start=(j == 0), stop=(j == NSC - 1))
            ctx_row = sg.tile([1, 2 * H], f32)
            nc.vector.tensor_copy(ctx_row, pctx)
            for j in range(NSC):
                pcj = ps_col.tile([128, 1], f32, tag="col")
                nc.tensor.matmul(
                    pcj, lhsT=ctx_row[0:1, j * 128:(j + 1) * 128], rhs=ones11,
                    start=True, stop=True)
                nc.vector.tensor_copy(xcol_sb[:, 1 + j:2 + j], pcj)

            # ---- LSTM gates (bias added in-PSUM via a K=1 matmul) ----
            pgs = []
            for ng in range(2):
                ns = slice(ng * 512, (ng + 1) * 512)
                pg = ps_row.tile([1, 512], f32, tag="row")
                for kc in range(5):
                    nc.tensor.matmul(
                        pg, lhsT=xcol_sb[:, kc:kc + 1], rhs=wihT_sb[:, kc, ns],
                        start=(kc == 0), stop=False)
                for kh in range(2):
                    nc.tensor.matmul(
                        pg, lhsT=h0col_sb[:, kh:kh + 1], rhs=whhT_sb[:, kh, ns],
                        start=False, stop=False)
                nc.tensor.matmul(
                    pg, lhsT=ones11, rhs=brow_sb[0:1, ns],
                    start=False, stop=True)
                pgs.append(pg)

            si = sg.tile([1, H], f32)
            sf = sg.tile([1, H], f32)
            tg = sg.tile([1, H], f32)
            so = sg.tile([1, H], f32)
            cnew_row = sg.tile([1, H], f32)
            nc.scalar.activation(out=si, in_=pgs[0][0:1, 0:H], func=AF.Sigmoid)
            nc.scalar.activation(out=sf, in_=pgs[0][0:1, H:2 * H], func=AF.Sigmoid)
            nc.scalar.activation(out=tg, in_=pgs[1][0:1, 0:H], func=AF.Tanh)
            nc.scalar.activation(out=so, in_=pgs[1][0:1, H:2 * H], func=AF.Sigmoid)
            nc.vector.tensor_mul(sf, sf, c0row_sb)
            nc.vector.tensor_mul(si, si, tg)
            nc.vector.tensor_add(cnew_row, sf, si)
            nc.scalar.dma_start(out=cnew_o, in_=cnew_row)
            nc.scalar.activation(out=tg, in_=cnew_row, func=AF.Tanh)
            nc.vector.tensor_mul(so, so, tg)
            nc.scalar.dma_start(out=hnew_o, in_=so)

            # ---- p_copy = w @ src: 65.5 MB fp32 read, cast to fp16 in-DMA
            # (SWDGE/gpsimd ring), fp16 matmuls at full PE rate ----
            src_t = src.rearrange("(c p) v -> p c v", p=128)
            chunks = [(i * VW, VW) for i in range(NVC - 1)]
            tw = VW // 4
            chunks += [((NVC - 1) * VW + i * tw, tw) for i in range(4)]
            for off, width in chunks:
                vs = slice(off, off + width)
                t = sp.tile([128, NSC, VW], f16, tag="src")
                nc.gpsimd.dma_start(out=t[:, 0:2, 0:width], in_=src_t[:, 0:2, vs])
                nc.gpsimd.dma_start(out=t[:, 2:4, 0:width], in_=src_t[:, 2:4, vs])
                prow = op.tile([1, VW], f32, tag="prow")
                for n in range(width // NW):
                    pt = ps_p.tile([1, NW], f32, tag="pc")
                    for sc in range(NSC):
                        nc.tensor.matmul(
                            pt, lhsT=wcolh_sb[:, sc:sc + 1],
                            rhs=t[:, sc, n * NW:(n + 1) * NW],
                            start=(sc == 0), stop=(sc == NSC - 1))
                    nc.vector.tensor_scalar_add(
                        prow[0:1, n * NW:(n + 1) * NW], pt, EPS)
                nc.scalar.dma_start(out=pcopy_o[0:1, vs], in_=prow[0:1, 0:width])

    nc.compile()
    return nc


def _tile4(x):
    """[C*128, N] -> [128, C, N] partition-chunked layout."""
    c = x.shape[0] // 128
    return np.ascontiguousarray(
        x.reshape(c, 128, *x.shape[1:]).transpose(1, 0, *range(2, x.ndim + 1)))


def _prep(inputs):
    f = lambda k: np.ascontiguousarray(np.asarray(inputs[k], dtype=np.float32))
    prev_word = np.asarray(inputs["prev_word"])
    state_h, state_c = f("state_h"), f("state_c")
    enc_all, cov_all, src_all = f("encoder_outputs"), f("coverage"), f("src_map")
    embed = f("embed")
    attn_W, attn_b, attn_v = f("attn_W"), f("attn_b"), f("attn_v")
    W_ih, W_hh = f("W_ih"), f("W_hh")
    b_ih, b_hh = f("b_ih"), f("b_hh")

    # replicated parameter repacks
    wencT = _tile4(np.ascontiguousarray(attn_W[:, H:3 * H].T))    # [128,4,256]
    wdecT = _tile4(np.ascontiguousarray(attn_W[:, 0:H].T))        # [128,2,256]
    auglhsT = np.ascontiguousarray(
        np.stack([attn_W[:, 3 * H:4 * H].sum(axis=1), attn_b]))   # [2,256]
    vcol = np.ascontiguousarray(attn_v.reshape(2, 128).T)         # [128,2]
    wihT = _tile4(np.ascontiguousarray(W_ih.T))                   # [128,5,1024]
    whhT = _tile4(np.ascontiguousarray(W_hh.T))                   # [128,2,1024]
    brow = np.ascontiguousarray((b_ih + b_hh).reshape(1, 4 * H))

    ones_s = np.ones((S,), np.float32)
    in_maps = []
    for b in range(B):
        enc_b = enc_all[b]
        in_maps.append({
            "src": src_all[b],
            "enc": _tile4(enc_b),
            "augrhs": np.ascontiguousarray(np.stack([cov_all[b], ones_s])),
            "h0col": np.ascontiguousarray(state_h[0, b].reshape(2, 128).T),
            "c0row": np.ascontiguousarray(state_c[0, b].reshape(1, H)),
            "embcol": np.ascontiguousarray(
                embed[int(prev_word[b, 0])].reshape(128, 1)),
            "wencT": wencT, "wdecT": wdecT, "auglhsT": auglhsT,
            "vcol": vcol, "wihT": wihT, "whhT": whhT, "brow": brow,
        })
    return in_maps


def _run(in_maps, **kw):
    from concourse import bass_utils
    if "nc" not in _cache:
        _cache["nc"] = _build_nc()
    return bass_utils.run_bass_kernel_spmd(
        _cache["nc"], in_maps, core_ids=list(range(B)), **kw)


def kernel(**inputs):
    res = _run(_prep(inputs))
    outs = res.results
    p_copy = np.concatenate([o["pcopy"] for o in outs], axis=0)
    attnw = np.concatenate([o["attnw"] for o in outs], axis=0)
    newcov = np.concatenate([o["newcov"] for o in outs], axis=0)
    hnew = np.concatenate([o["hnew"] for o in outs], axis=0)[None]
    cnew = np.concatenate([o["cnew"] for o in outs], axis=0)[None]
    return p_copy, attnw, newcov, (hnew, cnew)
